# revision 14
# baseline (speedup 1.0000x reference)
"""GAT 3-layer kernel for TRN2, 8 NeuronCores.

Sharding: edges by dst-owner core (12500 nodes/core). Per layer:
  GEMM (data-parallel) -> AllGather node table -> per-edge indirect gather
  -> ACT softmax scalars -> identity-matmul aggregation in PSUM.
Host does all index prep (degree-sorted windows, shared layout across cores).
"""

import numpy as np
import ml_dtypes

import concourse.bass as bass
import concourse.bacc as bacc
import concourse.mybir as mybir
from concourse import tile
from concourse.bass_utils import run_bass_kernel_spmd
from concourse.masks import make_identity

N = 100000
NCORES = 8
NPC = N // NCORES            # 12500 nodes per core
P = 128
NW = (NPC + P - 1) // P      # 98 windows
NPC_PAD = NW * P             # 12544
PAD_ROW = NCORES * NPC_PAD   # 100352 -> pad row index in full table
NEG = -1.0e30
FB = 8                       # f-batch per aggregation matmul

F32 = mybir.dt.float32
BF16 = mybir.dt.bfloat16
I32 = mybir.dt.int32


def _host_prep(x, edge_index, Ws, a_srcs, a_dsts):
    src = np.asarray(edge_index[0], dtype=np.int64)
    dst = np.asarray(edge_index[1], dtype=np.int64)
    loops = np.arange(N, dtype=np.int64)
    src = np.concatenate([src, loops])
    dst = np.concatenate([dst, loops])
    deg = np.bincount(dst, minlength=N)

    # per-core degree-sorted node order
    orders = []
    rank = np.empty(N, dtype=np.int64)     # node -> local rank in its core
    for k in range(NCORES):
        dk = deg[k * NPC:(k + 1) * NPC]
        o = np.argsort(-dk, kind="stable")
        orders.append(o)
        rank[k * NPC + o] = np.arange(NPC)

    # per-(core,window) span; shared across cores
    Lw = np.zeros((NCORES, NW), dtype=np.int64)
    for k in range(NCORES):
        dk_sorted = deg[k * NPC + orders[k]]
        dk_pad = np.concatenate([dk_sorted, np.zeros(NPC_PAD - NPC, dtype=np.int64)])
        Lw[k] = dk_pad.reshape(NW, P).max(axis=1)
    Lw_sh = Lw.max(axis=0)
    Lw_sh = np.maximum(Lw_sh, 1)
    colbase = np.concatenate([[0], np.cumsum(Lw_sh)])
    slots = int(colbase[-1])

    # global table row of a node
    g_row = (np.arange(N) // NPC) * NPC_PAD + rank

    # fill gather index array per core
    core_of = dst // NPC
    gidx = np.full((NCORES, P, slots), PAD_ROW, dtype=np.int32)
    for k in range(NCORES):
        m = core_of == k
        es, ed = src[m], dst[m]
        r = rank[ed]                      # local rank of dst
        order_e = np.argsort(r, kind="stable")
        es, r = es[order_e], r[order_e]
        # f = position within segment
        seg_start = np.searchsorted(r, np.arange(NPC))
        f = np.arange(len(r)) - seg_start[r]
        w, p = r // P, r % P
        cols = colbase[w] + f
        gidx[k, p, cols] = g_row[es].astype(np.int32)

    # superblocks: group windows so each SB spans <= SB_SLOTS slots
    SB_SLOTS = max(int(Lw_sh.max()), 224)
    sbs = []
    wstart = 0
    while wstart < NW:
        wend = wstart + 1
        while wend < NW and colbase[wend + 1] - colbase[wstart] <= SB_SLOTS:
            wend += 1
        sbs.append((wstart, wend))
        wstart = wend
    sb_max = max(int(colbase[e] - colbase[s]) for s, e in sbs)

    # x per core, transposed + padded
    xT = np.zeros((NCORES, 55, NPC_PAD), dtype=np.float32)
    for k in range(NCORES):
        xT[k, :, :NPC] = x[k * NPC + orders[k]].T

    # extended weights
    W_exts = []
    for W, a_s, a_d in zip(Ws, a_srcs, a_dsts):
        W_exts.append(np.concatenate([W, W @ a_s[0][:, None], W @ a_d[0][:, None]],
                                     axis=1).astype(np.float32))
    return dict(orders=orders, Lw_sh=Lw_sh.astype(int), colbase=colbase.astype(int),
                slots=slots, gidx=gidx, sbs=sbs, sb_max=sb_max, xT=xT, W_exts=W_exts)


def _build_program(prep, biases):
    Lw_sh, colbase, slots = prep["Lw_sh"], prep["colbase"], prep["slots"]
    sbs, sb_max = prep["sbs"], prep["sb_max"]
    CINS = [55, 32, 16]
    COUTS = [32, 16, 2]
    CMAX = 32

    nc = bacc.Bacc(None, target_bir_lowering=False, num_devices=NCORES)
    xT_in = nc.dram_tensor("xT", [55, NPC_PAD], F32, kind="ExternalInput")
    gidx_in = nc.dram_tensor("gidx", [P, slots], I32, kind="ExternalInput")
    w_ins = [nc.dram_tensor(f"W{l}", [CINS[l], COUTS[l] + 2], F32, kind="ExternalInput")
             for l in range(3)]
    b_ins = [nc.dram_tensor(f"b{l}", [P, COUTS[l]], F32, kind="ExternalInput")
             for l in range(3)]
    out_t = nc.dram_tensor("out", [NPC_PAD, 2], F32, kind="ExternalOutput")

    tbl_selfs = [nc.dram_tensor(f"tbls{l}", [NPC_PAD, COUTS[l] + 2], F32)
                 for l in range(3)]
    tbl_fulls = [nc.dram_tensor(f"tblf{l}", [PAD_ROW + 1, COUTS[l] + 2], F32,
                                addr_space="Shared") for l in range(3)]

    with tile.TileContext(nc) as tc:
        with (
            tc.tile_pool(name="const", bufs=1) as cpool,
            tc.tile_pool(name="gemm", bufs=2) as gpool,
            tc.tile_pool(name="gath", bufs=2) as gapool,
            tc.tile_pool(name="scal", bufs=1) as spool,
            tc.tile_pool(name="psum", bufs=2, space="PSUM") as ppool,
        ):
            ident = cpool.tile([P, P], F32)
            make_identity(nc, ident[:])
            ident_b = cpool.tile([P, P], BF16)
            nc.vector.tensor_copy(ident_b[:], ident[:])

            gidx_sb = cpool.tile([P, slots], I32)
            nc.sync.dma_start(gidx_sb[:], gidx_in[:, :])

            xT_slab = cpool.tile([55, NPC_PAD], F32, tag="xtslab")
            nc.sync.dma_start(xT_slab[:], xT_in[:, :])
            xT_next = cpool.tile([CMAX, NPC_PAD], F32, tag="xtnext")

            w_sb = []
            b_sb = []
            for l in range(3):
                wt = cpool.tile([CINS[l], COUTS[l] + 2], F32, tag=f"w{l}")
                nc.sync.dma_start(wt[:], w_ins[l][:, :])
                w_sb.append(wt)
                bt = cpool.tile([P, COUTS[l]], F32, tag=f"b{l}")
                nc.sync.dma_start(bt[:], b_ins[l][:, :])
                b_sb.append(bt)

            al_d = cpool.tile([P, NW], F32, tag="ald")

            for l in range(3):
                Cin, Cout = CINS[l], COUTS[l]
                Ct = Cout + 2
                xsrc = xT_slab if l == 0 else xT_next

                # pad row of table: h cols = 0, al cols = NEG
                padr = gpool.tile([1, Ct], F32, tag="padr")
                nc.gpsimd.memset(padr[:, :Cout], 0.0)
                nc.gpsimd.memset(padr[:, Cout:], NEG)
                nc.sync.dma_start(tbl_fulls[l][PAD_ROW:PAD_ROW + 1, :], padr[:])

                # ---- GEMM + table build, per 128-node chunk ----
                for w in range(NW):
                    ht_ps = ppool.tile([Ct, P], F32, tag="htps")
                    nc.tensor.matmul(ht_ps[:], lhsT=w_sb[l][:Cin, :],
                                     rhs=xsrc[:Cin, w * P:(w + 1) * P],
                                     start=True, stop=True)
                    ht_sb = gpool.tile([Ct, P], F32, tag="htsb")
                    nc.vector.tensor_copy(ht_sb[:], ht_ps[:])
                    h_ps = ppool.tile([P, Ct], F32, tag="hps")
                    nc.tensor.transpose(h_ps[:], ht_sb[:], ident[:Ct, :Ct])
                    tb = gpool.tile([P, Ct], F32, tag="tb")
                    nc.vector.tensor_copy(tb[:, :Cout + 1], h_ps[:, :Cout + 1])
                    # al_s residual: lo = f32(al) - bf16(hi)
                    nc.vector.tensor_tensor(tb[:, Cout + 1:Cout + 2],
                                            h_ps[:, Cout:Cout + 1],
                                            tb[:, Cout:Cout + 1],
                                            op=mybir.AluOpType.subtract)
                    nc.vector.tensor_copy(al_d[:, w:w + 1], h_ps[:, Cout + 1:Cout + 2])
                    nc.sync.dma_start(tbl_selfs[l][w * P:(w + 1) * P, :], tb[:])

                # ---- AllGather table ----
                nc.gpsimd.collective_compute(
                    "AllGather", mybir.AluOpType.bypass,
                    replica_groups=[list(range(NCORES))],
                    ins=[tbl_selfs[l].ap().opt()],
                    outs=[tbl_fulls[l][:PAD_ROW, :].opt()],
                )

                # ---- per superblock: gather + softmax + aggregate ----
                for (ws, we) in sbs:
                    c0, c1 = int(colbase[ws]), int(colbase[we])
                    nsl = c1 - c0
                    G = gapool.tile([P, sb_max * Ct], F32, tag="G")
                    Gv = G[:].rearrange("p (s c) -> p s c", c=Ct)
                    for j in range(nsl):
                        nc.gpsimd.indirect_dma_start(
                            out=Gv[:, j, :],
                            out_offset=None,
                            in_=tbl_fulls[l][:, :],
                            in_offset=bass.IndirectOffsetOnAxis(
                                ap=gidx_sb[:, c0 + j:c0 + j + 1], axis=0),
                        )
                    S = spool.tile([P, sb_max], F32, tag="S")
                    nc.vector.tensor_tensor(S[:, :nsl], Gv[:, :nsl, Cout],
                                            Gv[:, :nsl, Cout + 1],
                                            op=mybir.AluOpType.add)
                    EE = spool.tile([P, sb_max], F32, tag="EE")
                    dn = spool.tile([P, NW], F32, tag="dn")
                    for w in range(ws, we):
                        a0 = int(colbase[w]) - c0
                        a1 = int(colbase[w + 1]) - c0
                        nc.scalar.activation(EE[:, a0:a1], S[:, a0:a1],
                                             mybir.ActivationFunctionType.Lrelu,
                                             bias=al_d[:, w:w + 1], alpha=0.2)
                        nc.scalar.activation(EE[:, a0:a1], EE[:, a0:a1],
                                             mybir.ActivationFunctionType.Exp)
                        nc.vector.tensor_reduce(dn[:, w:w + 1], EE[:, a0:a1],
                                                axis=mybir.AxisListType.X,
                                                op=mybir.AluOpType.add)
                    rdn = spool.tile([P, NW], F32, tag="rdn")
                    nc.vector.tensor_scalar_add(rdn[:, ws:we], dn[:, ws:we], 1e-38)
                    nc.vector.reciprocal(rdn[:, ws:we], rdn[:, ws:we])
                    A = spool.tile([P, sb_max], BF16, tag="A")
                    for w in range(ws, we):
                        a0 = int(colbase[w]) - c0
                        a1 = int(colbase[w + 1]) - c0
                        nc.vector.tensor_scalar_mul(A[:, a0:a1], EE[:, a0:a1],
                                                    rdn[:, w:w + 1])
                    M = gapool.tile([P, sb_max * Cout], BF16, tag="M")
                    Mv = M[:].rearrange("p (s c) -> p s c", c=Cout)
                    Av = A[:, :nsl].rearrange("p (s o) -> p s o", o=1).to_broadcast(
                        [P, nsl, Cout])
                    nc.vector.tensor_tensor(Mv[:, :nsl, :Cout], Gv[:, :nsl, :Cout],
                                            Av, op=mybir.AluOpType.mult)

                    for w in range(ws, we):
                        a0 = int(colbase[w]) - c0
                        L = int(Lw_sh[w])
                        nfb = (L + FB - 1) // FB
                        ag_ps = ppool.tile([P, FB * CMAX], F32, tag="agps")
                        for fb in range(nfb):
                            f0 = fb * FB
                            nf = min(FB, L - f0)
                            nc.tensor.matmul(
                                ag_ps[:, :nf * Cout], lhsT=ident_b[:],
                                rhs=Mv[:, a0 + f0:a0 + f0 + nf, :].rearrange(
                                    "p s c -> p (s c)"),
                                start=(fb == 0), stop=(fb == nfb - 1))
                        # fold nf partial columns in psum down to [P, Cout]
                        acc = gpool.tile([P, CMAX], F32, tag="acc")
                        nc.vector.tensor_copy(acc[:, :Cout], ag_ps[:, :Cout])
                        for j in range(1, FB):
                            nc.vector.tensor_tensor(
                                acc[:, :Cout], acc[:, :Cout],
                                ag_ps[:, j * Cout:(j + 1) * Cout],
                                op=mybir.AluOpType.add)
                        nc.vector.tensor_tensor(acc[:, :Cout], acc[:, :Cout],
                                                b_sb[l][:, :Cout],
                                                op=mybir.AluOpType.add)
                        if l < 2:
                            nc.vector.tensor_scalar_max(acc[:, :Cout],
                                                        acc[:, :Cout], 0.0)
                            tr_ps = ppool.tile([CMAX, P], F32, tag="trps")
                            nc.tensor.transpose(tr_ps[:Cout, :], acc[:, :Cout],
                                                ident[:])
                            nc.vector.tensor_copy(
                                xT_next[:Cout, w * P:(w + 1) * P], tr_ps[:Cout, :])
                        else:
                            # log_softmax over 2 cols
                            mx = gpool.tile([P, 1], F32, tag="mx")
                            nc.vector.tensor_reduce(mx[:], acc[:, :2],
                                                    axis=mybir.AxisListType.X,
                                                    op=mybir.AluOpType.max)
                            t = gpool.tile([P, 2], F32, tag="t")
                            nc.vector.tensor_scalar(t[:], acc[:, :2], mx[:], None,
                                                    op0=mybir.AluOpType.subtract)
                            ex = gpool.tile([P, 2], F32, tag="ex")
                            nc.scalar.activation(ex[:], t[:],
                                                 mybir.ActivationFunctionType.Exp)
                            sm = gpool.tile([P, 1], F32, tag="sm")
                            nc.vector.tensor_reduce(sm[:], ex[:],
                                                    axis=mybir.AxisListType.X,
                                                    op=mybir.AluOpType.add)
                            lsm = gpool.tile([P, 1], F32, tag="lsm")
                            nc.scalar.activation(lsm[:], sm[:],
                                                 mybir.ActivationFunctionType.Ln)
                            res = gpool.tile([P, 2], F32, tag="res")
                            nc.vector.tensor_scalar(res[:], t[:], lsm[:], None,
                                                    op0=mybir.AluOpType.subtract)
                            nc.sync.dma_start(out_t[w * P:(w + 1) * P, :], res[:])
    nc.compile()
    return nc


def kernel(x, edge_index, W1, a_src1, a_dst1, b1, W2, a_src2, a_dst2, b2,
           W3, a_src3, a_dst3, b3):
    x = np.asarray(x, dtype=np.float32)
    Ws = [np.asarray(W1, np.float32), np.asarray(W2, np.float32),
          np.asarray(W3, np.float32)]
    a_srcs = [np.asarray(a, np.float32) for a in (a_src1, a_src2, a_src3)]
    a_dsts = [np.asarray(a, np.float32) for a in (a_dst1, a_dst2, a_dst3)]
    bs = [np.asarray(b, np.float32) for b in (b1, b2, b3)]

    prep = _host_prep(x, edge_index, Ws, a_srcs, a_dsts)
    nc = _build_program(prep, bs)

    in_maps = []
    for k in range(NCORES):
        im = {"xT": prep["xT"][k], "gidx": prep["gidx"][k]}
        for l in range(3):
            im[f"W{l}"] = prep["W_exts"][l]
            im[f"b{l}"] = np.tile(bs[l][None, :], (P, 1)).astype(np.float32)
        in_maps.append(im)

    res = run_bass_kernel_spmd(nc, in_maps, core_ids=list(range(NCORES)))
    out = np.empty((N, 2), dtype=np.float32)
    for k in range(NCORES):
        out[k * NPC + prep["orders"][k]] = res.results[k]["out"][:NPC]
    return out
